# revision 26
# baseline (speedup 1.0000x reference)
"""GRU-style segmented-scan aggregator for Trainium2 (8 NeuronCores).

Reference computes, per node n with messages m_1..m_L sorted by time t:
    h <- W @ (m + h) + b   starting from h = 0
and returns the final h per node (zeros for empty nodes).

Because every step uses the SAME matrix W, the final state has the closed
form (h_0 = 0):
    h = sum_{k=0}^{L-1} W^{k+1} m_{(L-1-k)}  +  S_L b,   S_L = sum_{p<L} W^p
i.e. the k-th message FROM THE END is hit by W^{k+1}.  This turns the
sequential scan into independent batched matmuls against precomputed powers
of W -- ideal for the PE array.

Device layout (per core, SPMD over 8 cores):
  - nodes are sorted by message count (desc) and dealt round-robin to cores;
    each core owns <=1024 node slots, columns of a [256 feat x 1024] H^T
    accumulator kept in PSUM (2 chunks of 128 partitions).
  - the bias term S_L b (+ host-folded tail) is accumulated FIRST via an
    identity matmul, so a PSUM column is final right after the last message
    matmul that touches it; since n_k shrinks with k, high columns finish
    early and are cast + written back while the tail steps still run --
    this hides the HBM write's long completion latency.
  - step k multiplies W^{k+1} (lhsT, 4 chunks of 128x128) with the k-th-from-
    end messages of the first n_k slots (rhs, features on partitions), and
    accumulates into PSUM.
  - all PE operands are fp16 (1 cycle/row on the PE vs 4 for fp32, and half
    the HBM traffic); PSUM accumulation stays fp32.  fp16 rounding gives
    ~4e-4 relative error, far inside the 2e-2 gate.
  - step blocks are merged into >=~500KB superblock DMAs (big transfers are
    what the DMA engines like), and the DMA triggers alternate between the
    two HWDGE rings (Sync + Scalar sequencers) so trigger dispatch (~0.7us
    each) is not serialized on one queue.

Host does the (cheap) data marshalling: lexsort by (index, t), gather into
the k-major column layout, precompute W powers in fp64, scatter results back.
Steps with fewer than TAIL_MIN live node slots are folded into the per-node
bias term on the host: a 512-col weight DMA is not worth a handful of
message columns.
"""

import numpy as np

import concourse.bass as bass
import concourse.mybir as mybir
from concourse import tile
from concourse.bass_utils import run_bass_kernel_spmd
import bass_rust

_N_PROCS = 27


class _SplitDrainTC(tile.TileContext):
    """TileContext whose kernel-tail drain is split into one drain per proc.

    The walrus build in this container rejects instructions carrying more
    than one sync wait; the stock tail drain waits on every proc at once.
    Emitting a chain of drains, each waiting on a single semaphore, is
    semantically identical (all procs quiesced before the exit barrier).
    """

    def _drain_and_barrier(self, tick_clock, wait_clock):
        gc = tick_clock.global_clock
        for p in range(_N_PROCS):
            if gc[p] <= 0:
                continue
            d = self.nc.sync.drain()
            vc = bass_rust.VectorClock(
                [gc[q] if q == p else 0 for q in range(_N_PROCS)])
            wait_clock.add_sem_waits(d.ins, bass_rust.ScopedClock({None: vc}))
        assert self.sems is not None
        popped = self.nc._tile_sem_poison_stack.pop()
        assert popped is self._sem_poison
        self.nc.all_engine_barrier()
        self.nc.clear_and_free_semaphores(list(self.sems.allocated().values()))
        self.nc.all_engine_barrier()

N_CORES = 8
DIM = 256
SLOTS = 1024    # node slots per core == PSUM accumulator width
TAIL_MIN = 256  # fold steps with fewer live slots into the host bias term
SB_COLS = 2048  # superblock DMA target size (f16 cols; 2048 ~= 512KB)

USE_SCALAR_DMA = True   # alternate input DMAs onto the Activation HWDGE ring
EARLY_WB = True         # write back PSUM regions as soon as they finalize

_NC_CACHE: dict = {}


def _groups(n_k):
    """Greedy superblock grouping: consecutive steps until >= SB_COLS cols."""
    groups, cur, cols = [], [], 0
    for k in range(len(n_k)):
        cur.append(k)
        cols += 512 + 2 * n_k[k]
        if cols >= SB_COLS:
            groups.append(cur)
            cur, cols = [], 0
    if cur:
        groups.append(cur)
    return groups


def _build_nc(K0: int, n_k: tuple, Cdev: int):
    """Build the Bass program for one core (shared by all 8 via SPMD).

    This walrus build accepts at most ONE sync wait per instruction, so the
    kernel is written with zero SBUF-slot reuse (every stream block gets its
    own tile; a reused slot would need WAR+WAW = 2 waits on its DMA) and the
    bias is injected via identity matmul instead of a DVE add (which would
    carry PE + DMA = 2 waits).
    """
    f16 = mybir.dt.float16
    f32 = mybir.dt.float32
    nc = bass.Bass()

    groups = _groups(n_k)
    # last step that touches PSUM cols >= a  (bank [a:a+512) final after it)
    def k_fin(a):
        ks = [k for k in range(K0) if n_k[k] > a]
        return max(ks) if ks else -1
    # writeback regions = whole PSUM banks, keyed by finalize step
    regions = sorted([(k_fin(a), a, a + 512)
                      for a in (0, 512) if n_k[0] > a])

    # stream: per-(bank, feature-chunk) bias blocks [128 ident | 512 bt] (the
    # identity is duplicated so each bias matmul carries ONE tile dep and the
    # first PE instruction waits on a minimal 164KB DMA),
    # then superblocks of per-step [512 weight cols | n_k hi | n_k lo]
    banks = sorted({a for _, a, _ in regions})
    FB1 = 128 + 1024
    FB = FB1 * len(banks)
    Q = FB + K0 * 512 + 2 * Cdev
    mw = nc.dram_tensor("mw", [128, Q], f16, kind="ExternalInput")
    out = nc.dram_tensor("out", [128, 2 * SLOTS], f16, kind="ExternalOutput")

    with _SplitDrainTC(nc) as tc:
        with (
            tc.tile_pool(name="m", bufs=1) as mpool,
            tc.tile_pool(name="misc", bufs=1) as miscpool,
            tc.tile_pool(name="ps", bufs=1, space="PSUM") as pspool,
        ):
            # one PSUM tile per (feature chunk i, 512-col bank) so each bank
            # closes and writes back independently
            phs = {(i, s): pspool.tile([128, 512], f32, tag=f"ph{i}_{s}",
                                       name=f"ph{i}_{s}")
                   for i in range(2) for s in (0, 512) if n_k[0] > s}

            # bias first (start=True on every PSUM bank) via identity matmul;
            # one small DMA per bank so the PE can start sooner
            for bi, s in enumerate(banks):
                fb = mpool.tile([128, FB1], f16, tag=f"fb{s}", name=f"fb{s}")
                feng = (nc.scalar if (USE_SCALAR_DMA and bi % 2 == 1)
                        else nc.sync)
                feng.dma_start(fb[:], mw[:, bi * FB1:(bi + 1) * FB1])
                for i in range(2):
                    nc.tensor.matmul(
                        phs[i, s][:], fb[:, 0:128],
                        fb[:, 128 + i * 512: 128 + (i + 1) * 512],
                        start=True, stop=False, skip_group_check=True,
                    )

            done = set()          # regions already written back
            q = FB
            for g, ks in enumerate(groups):
                blk = sum(512 + 2 * n_k[k] for k in ks)
                mk = mpool.tile([128, blk], f16, tag=f"mk{g}", name=f"mk{g}")
                eng = nc.sync if (g % 2 == 0 or not USE_SCALAR_DMA) else nc.scalar
                eng.dma_start(mk[:], mw[:, q:q + blk])
                q += blk
                o = 0
                for k in ks:
                    nk = n_k[k]
                    for i in range(2):      # output feature chunk (PSUM partitions)
                        for j in range(2):  # contraction chunk
                            wt = mk[:, o + j * 256 + i * 128:
                                    o + j * 256 + (i + 1) * 128]
                            for s in range(0, nk, 512):
                                e = min(nk, s + 512)
                                # close the bank's accumulation group on its
                                # last matmul
                                stop = (j == 1 and k == k_fin(s))
                                nc.tensor.matmul(
                                    phs[i, s][:, 0:e - s], wt,
                                    mk[:, o + 512 + j * nk + s:
                                        o + 512 + j * nk + e],
                                    start=False, stop=stop,
                                    skip_group_check=True,
                                )
                    o += 512 + 2 * nk
                    # early writeback: banks whose last touch was step k.
                    # both feature chunks of a bank share one SBUF tile and
                    # one DMA so the HBM write has 2KB-contiguous rows
                    # (4 separate 1KB-row writes ran at ~30GB/s).
                    for kf, a, b in regions:
                        if not EARLY_WB:
                            break
                        if kf == k and (a, b) not in done:
                            done.add((a, b))
                            ot = miscpool.tile([128, 2 * 512], f16,
                                               tag=f"ot_{a}", name=f"ot_{a}")
                            for i in range(2):
                                nc.vector.tensor_copy(
                                    ot[:, i * 512:(i + 1) * 512], phs[i, a][:])
                            oc = (a // 512) * 1024
                            nc.gpsimd.dma_start(out[:, oc:oc + 1024], ot[:])
            # banks not flushed early (EARLY_WB off, or no finalize step)
            for kf, a, b in regions:
                if (a, b) not in done:
                    ot = miscpool.tile([128, 2 * 512], f16,
                                       tag=f"otz_{a}", name=f"otz_{a}")
                    for i in range(2):
                        nc.vector.tensor_copy(
                            ot[:, i * 512:(i + 1) * 512], phs[i, a][:])
                    oc = (a // 512) * 1024
                    nc.gpsimd.dma_start(out[:, oc:oc + 1024], ot[:])
    return nc


def _prepare(msg, index, t, dim_size, W, b):
    """Host-side marshalling. Returns (in_maps, node_ids, schedule key)."""
    E, D = msg.shape
    counts = np.bincount(index, minlength=dim_size)
    order = np.lexsort((t, index))            # stable: primary index, secondary t
    msg_sorted = msg[order]                   # [E, D] grouped by node, t-ascending
    seg_starts = np.zeros(dim_size, np.int64)
    seg_starts[1:] = np.cumsum(counts)[:-1]

    nodesort = np.argsort(-counts, kind="stable")
    nz = nodesort[counts[nodesort] > 0]
    per_core = -(-len(nz) // N_CORES)
    assert per_core <= SLOTS, f"too many nodes per core: {per_core}"

    node_ids = np.full((N_CORES, SLOTS), -1, np.int64)
    for c in range(N_CORES):
        ids = nz[c::N_CORES]
        node_ids[c, :len(ids)] = ids
    cc = np.where(node_ids >= 0, counts[np.maximum(node_ids, 0)], 0)  # [8, SLOTS]

    Lmax = int(cc.max())
    n_k = tuple(int((cc > k).sum(axis=1).max()) for k in range(Lmax))

    # device handles steps k < K0; the tail (k >= K0, a small % of messages)
    # is folded into the per-node bias term on the host.  K0 stops where
    # fewer than TAIL_MIN slots are still live (weight DMA no longer pays
    # for itself); the SBUF guard keeps the no-reuse footprint bounded.
    K0 = Lmax
    while K0 > 1 and n_k[K0 - 1] < TAIL_MIN:
        K0 -= 1
    while K0 > 1 and (K0 * 512 + 2 * sum(n_k[:K0]) + 128 + 2 * SLOTS) > 85000:
        K0 -= 1
    Cdev = int(sum(n_k[:K0]))

    # column -> position in msg_sorted (or -1 = zero pad), k-major layout
    rowidx = np.full((N_CORES, Cdev), -1, np.int64)
    off = 0
    for k in range(K0):
        nk = n_k[k]
        nid = node_ids[:, :nk]
        ck = cc[:, :nk]
        active = k < ck
        pos = seg_starts[np.maximum(nid, 0)] + ck - 1 - k
        rowidx[:, off:off + nk] = np.where(active, pos, -1)
        off += nk

    # weights: powers of W in fp64, stored transposed (lhsT chunks).
    # wfull per k: cols [0,256) = (W^{k+1}).T rows 0:128 (j=0 chunk),
    #              cols [256,512) = rows 128:256 (j=1 chunk).
    Wd = W.astype(np.float64)
    bd = b.astype(np.float64)
    wfull = np.empty((128, K0 * 512), np.float16)
    s_table = np.zeros((Lmax + 1, D), np.float64)   # s_p = S_p b
    Wpows = []                                      # W^{k+1} (fp64), k = 0..Lmax-1
    P = Wd.copy()
    for k in range(Lmax):
        if k < K0:
            WT = P.T.astype(np.float16)             # (W^{k+1}).T
            wfull[:, k * 512:k * 512 + 256] = WT[:128, :]
            wfull[:, k * 512 + 256:(k + 1) * 512] = WT[128:, :]
        Wpows.append(P)
        s_table[k + 1] = Wd @ s_table[k] + bd
        P = P @ Wd

    # per-(core, slot) bias term: S_L b plus host-folded tail contributions
    bterm = s_table[cc]                              # [8, SLOTS, 256] fp64
    for k in range(K0, Lmax):
        nk = n_k[k]
        act = k < cc[:, :nk]                         # [8, nk]
        cs, ss = np.nonzero(act)
        pos = seg_starts[node_ids[cs, ss]] + cc[cs, ss] - 1 - k
        Y = msg_sorted[pos].astype(np.float64) @ Wpows[k].T
        bterm[cs, ss] += Y
    bterm16 = bterm.astype(np.float16)

    ident = np.zeros((128, 128), np.float16)
    np.fill_diagonal(ident, 1.0)

    banks = [0] + ([512] if n_k[0] > 512 else [])
    FB1 = 128 + 1024
    FB = FB1 * len(banks)
    Q = FB + K0 * 512 + 2 * Cdev
    in_maps = []
    for c in range(N_CORES):
        ri = rowidx[c]
        Mg = msg_sorted[np.maximum(ri, 0)].astype(np.float16)
        Mg[ri < 0] = 0.0                             # [Cdev, 256]
        hi = Mg[:, :128].T                           # [128, Cdev]
        lo = Mg[:, 128:].T
        mwb = np.empty((128, Q), np.float16)
        for bi, s in enumerate(banks):
            o = bi * FB1
            mwb[:, o:o + 128] = ident
            mwb[:, o + 128:o + 640] = bterm16[c, s:s + 512, :128].T
            mwb[:, o + 640:o + FB1] = bterm16[c, s:s + 512, 128:].T
        off = 0
        q = FB
        for k in range(K0):
            nk = n_k[k]
            mwb[:, q:q + 512] = wfull[:, k * 512:(k + 1) * 512]
            mwb[:, q + 512:q + 512 + nk] = hi[:, off:off + nk]
            mwb[:, q + 512 + nk:q + 512 + 2 * nk] = lo[:, off:off + nk]
            off += nk
            q += 512 + 2 * nk
        in_maps.append({"mw": mwb})
    return in_maps, node_ids, (K0, n_k[:K0], Cdev)


def _run(inputs: dict, trace: bool = False, **run_kwargs):
    msg = np.ascontiguousarray(np.asarray(inputs["msg"], dtype=np.float32))
    index = np.asarray(inputs["index"]).astype(np.int64)
    t = np.asarray(inputs["t"], dtype=np.float32)
    W = np.asarray(inputs["W"], dtype=np.float32)
    b = np.asarray(inputs["b"], dtype=np.float32)
    dim_size = int(inputs["dim_size"])

    in_maps, node_ids, key = _prepare(msg, index, t, dim_size, W, b)
    K0, n_k, Cdev = key
    if key not in _NC_CACHE:
        _NC_CACHE[key] = _build_nc(K0, n_k, Cdev)
    nc = _NC_CACHE[key]

    res = run_bass_kernel_spmd(nc, in_maps, list(range(N_CORES)),
                               trace=trace, **run_kwargs)

    hidden = np.zeros((dim_size, DIM), np.float32)
    for c in range(N_CORES):
        o = res.results[c]["out"].astype(np.float32)  # [128, 2*SLOTS]
        # out layout: [bank*1024 + i*512 + (slot - bank)] for bank in {0,512}
        hc = np.empty((SLOTS, DIM), np.float32)
        for bank in range(2):
            for i in range(2):
                blk = o[:, bank * 1024 + i * 512: bank * 1024 + (i + 1) * 512]
                hc[bank * 512:(bank + 1) * 512, i * 128:(i + 1) * 128] = blk.T
        valid = node_ids[c] >= 0
        hidden[node_ids[c][valid]] = hc[valid]
    return hidden, res


def kernel(**inputs) -> np.ndarray:
    hidden, _ = _run(inputs, trace=False)
    return hidden


# revision 28
# speedup vs baseline: 1.0499x; 1.0499x over previous
"""GRU-style segmented-scan aggregator for Trainium2 (8 NeuronCores).

Reference computes, per node n with messages m_1..m_L sorted by time t:
    h <- W @ (m + h) + b   starting from h = 0
and returns the final h per node (zeros for empty nodes).

Because every step uses the SAME matrix W, the final state has the closed
form (h_0 = 0):
    h = sum_{k=0}^{L-1} W^{k+1} m_{(L-1-k)}  +  S_L b,   S_L = sum_{p<L} W^p
i.e. the k-th message FROM THE END is hit by W^{k+1}.  This turns the
sequential scan into independent batched matmuls against precomputed powers
of W -- ideal for the PE array.

Device layout (per core, SPMD over 8 cores):
  - nodes are sorted by message count (desc) and dealt round-robin to cores;
    each core owns <=1024 node slots, columns of a [256 feat x 1024] H^T
    accumulator kept in PSUM (2 chunks of 128 partitions).
  - the bias term S_L b (+ host-folded tail) is accumulated FIRST via an
    identity matmul, so a PSUM column is final right after the last message
    matmul that touches it; since n_k shrinks with k, high columns finish
    early and are cast + written back while the tail steps still run --
    this hides the HBM write's long completion latency.
  - step k multiplies W^{k+1} (lhsT, 4 chunks of 128x128) with the k-th-from-
    end messages of the first n_k slots (rhs, features on partitions), and
    accumulates into PSUM.
  - all PE operands are fp16 (1 cycle/row on the PE vs 4 for fp32, and half
    the HBM traffic); PSUM accumulation stays fp32.  fp16 rounding gives
    ~4e-4 relative error, far inside the 2e-2 gate.
  - step blocks are merged into >=~500KB superblock DMAs (big transfers are
    what the DMA engines like), and the DMA triggers alternate between the
    two HWDGE rings (Sync + Scalar sequencers) so trigger dispatch (~0.7us
    each) is not serialized on one queue.

Host does the (cheap) data marshalling: lexsort by (index, t), gather into
the k-major column layout, precompute W powers in fp64, scatter results back.
Steps with fewer than TAIL_MIN live node slots are folded into the per-node
bias term on the host: a 512-col weight DMA is not worth a handful of
message columns.
"""

import numpy as np

import concourse.bass as bass
import concourse.mybir as mybir
from concourse import tile
from concourse.bass_utils import run_bass_kernel_spmd
import bass_rust

_N_PROCS = 27


class _SplitDrainTC(tile.TileContext):
    """TileContext whose kernel-tail drain is split into one drain per proc.

    The walrus build in this container rejects instructions carrying more
    than one sync wait; the stock tail drain waits on every proc at once.
    Emitting a chain of drains, each waiting on a single semaphore, is
    semantically identical (all procs quiesced before the exit barrier).
    """

    def _drain_and_barrier(self, tick_clock, wait_clock):
        gc = tick_clock.global_clock
        for p in range(_N_PROCS):
            if gc[p] <= 0:
                continue
            d = self.nc.sync.drain()
            vc = bass_rust.VectorClock(
                [gc[q] if q == p else 0 for q in range(_N_PROCS)])
            wait_clock.add_sem_waits(d.ins, bass_rust.ScopedClock({None: vc}))
        assert self.sems is not None
        popped = self.nc._tile_sem_poison_stack.pop()
        assert popped is self._sem_poison
        self.nc.all_engine_barrier()
        self.nc.clear_and_free_semaphores(list(self.sems.allocated().values()))
        self.nc.all_engine_barrier()

N_CORES = 8
DIM = 256
SLOTS = 1024    # node slots per core == PSUM accumulator width
TAIL_MIN = 256  # fold steps with fewer live slots into the host bias term
SB_COLS = 2048  # superblock DMA target size (f16 cols; 2048 ~= 512KB)

USE_SCALAR_DMA = True   # alternate input DMAs onto the Activation HWDGE ring
EARLY_WB = True         # write back PSUM regions as soon as they finalize
N_WARM = 10             # dummy matmuls that ramp the PE clock during startup

_NC_CACHE: dict = {}


def _groups(n_k):
    """Greedy superblock grouping: consecutive steps until >= SB_COLS cols."""
    groups, cur, cols = [], [], 0
    for k in range(len(n_k)):
        cur.append(k)
        cols += 512 + 2 * n_k[k]
        if cols >= SB_COLS:
            groups.append(cur)
            cur, cols = [], 0
    if cur:
        groups.append(cur)
    return groups


def _build_nc(K0: int, n_k: tuple, Cdev: int):
    """Build the Bass program for one core (shared by all 8 via SPMD).

    This walrus build accepts at most ONE sync wait per instruction, so the
    kernel is written with zero SBUF-slot reuse (every stream block gets its
    own tile; a reused slot would need WAR+WAW = 2 waits on its DMA) and the
    bias is injected via identity matmul instead of a DVE add (which would
    carry PE + DMA = 2 waits).
    """
    f16 = mybir.dt.float16
    f32 = mybir.dt.float32
    nc = bass.Bass()

    groups = _groups(n_k)
    # last step that touches PSUM cols >= a  (bank [a:a+512) final after it)
    def k_fin(a):
        ks = [k for k in range(K0) if n_k[k] > a]
        return max(ks) if ks else -1
    # writeback regions = whole PSUM banks, keyed by finalize step
    regions = sorted([(k_fin(a), a, a + 512)
                      for a in (0, 512) if n_k[0] > a])

    # stream: per-(bank, feature-chunk) bias blocks [128 ident | 512 bt] (the
    # identity is duplicated so each bias matmul carries ONE tile dep and the
    # first PE instruction waits on a minimal 164KB DMA),
    # then superblocks of per-step [512 weight cols | n_k hi | n_k lo]
    banks = sorted({a for _, a, _ in regions})
    FB1 = 128 + 1024
    FB = FB1 * len(banks)
    Q = FB + K0 * 512 + 2 * Cdev
    mw = nc.dram_tensor("mw", [128, Q], f16, kind="ExternalInput")
    out = nc.dram_tensor("out", [128, 2 * SLOTS], f16, kind="ExternalOutput")

    with _SplitDrainTC(nc) as tc:
        with (
            tc.tile_pool(name="m", bufs=1) as mpool,
            tc.tile_pool(name="misc", bufs=1) as miscpool,
            tc.tile_pool(name="ps", bufs=1, space="PSUM") as pspool,
        ):
            # one PSUM tile per (feature chunk i, 512-col bank) so each bank
            # closes and writes back independently
            phs = {(i, s): pspool.tile([128, 512], f32, tag=f"ph{i}_{s}",
                                       name=f"ph{i}_{s}")
                   for i in range(2) for s in (0, 512) if n_k[0] > s}

            # PE clock warm-up: the PE starts in a low DVFS pstate and takes
            # ~6us of continuous execution to reach full clock (the first
            # ~25 real matmuls otherwise run at ~462ns instead of ~217ns for
            # 512 rows).  Burn that ramp on dummy matmuls over a zeroed
            # scratch tile while the first input DMAs are still in flight;
            # the results go to a scratch PSUM bank and are never read.
            warm = miscpool.tile([128, 512], f16, tag="warm", name="warm")
            wps = pspool.tile([128, 512], f32, tag="wps", name="wps")
            nc.vector.memset(warm[:], 0.0)
            for _ in range(N_WARM):
                nc.tensor.matmul(wps[:], warm[:, 0:128], warm[:],
                                 start=True, stop=True,
                                 skip_group_check=True)

            # bias first (start=True on every PSUM bank) via identity matmul;
            # one small DMA per bank so the PE can start sooner
            for bi, s in enumerate(banks):
                fb = mpool.tile([128, FB1], f16, tag=f"fb{s}", name=f"fb{s}")
                feng = (nc.scalar if (USE_SCALAR_DMA and bi % 2 == 1)
                        else nc.sync)
                feng.dma_start(fb[:], mw[:, bi * FB1:(bi + 1) * FB1])
                for i in range(2):
                    nc.tensor.matmul(
                        phs[i, s][:], fb[:, 0:128],
                        fb[:, 128 + i * 512: 128 + (i + 1) * 512],
                        start=True, stop=False, skip_group_check=True,
                    )

            done = set()          # regions already written back
            q = FB
            for g, ks in enumerate(groups):
                blk = sum(512 + 2 * n_k[k] for k in ks)
                mk = mpool.tile([128, blk], f16, tag=f"mk{g}", name=f"mk{g}")
                eng = nc.sync if (g % 2 == 0 or not USE_SCALAR_DMA) else nc.scalar
                eng.dma_start(mk[:], mw[:, q:q + blk])
                q += blk
                o = 0
                for k in ks:
                    nk = n_k[k]
                    for i in range(2):      # output feature chunk (PSUM partitions)
                        for j in range(2):  # contraction chunk
                            wt = mk[:, o + j * 256 + i * 128:
                                    o + j * 256 + (i + 1) * 128]
                            for s in range(0, nk, 512):
                                e = min(nk, s + 512)
                                # close the bank's accumulation group on its
                                # last matmul
                                stop = (j == 1 and k == k_fin(s))
                                nc.tensor.matmul(
                                    phs[i, s][:, 0:e - s], wt,
                                    mk[:, o + 512 + j * nk + s:
                                        o + 512 + j * nk + e],
                                    start=False, stop=stop,
                                    skip_group_check=True,
                                )
                    o += 512 + 2 * nk
                    # early writeback: banks whose last touch was step k.
                    # both feature chunks of a bank share one SBUF tile and
                    # one DMA so the HBM write has 2KB-contiguous rows
                    # (4 separate 1KB-row writes ran at ~30GB/s).
                    for kf, a, b in regions:
                        if not EARLY_WB:
                            break
                        if kf == k and (a, b) not in done:
                            done.add((a, b))
                            ot = miscpool.tile([128, 2 * 512], f16,
                                               tag=f"ot_{a}", name=f"ot_{a}")
                            for i in range(2):
                                nc.vector.tensor_copy(
                                    ot[:, i * 512:(i + 1) * 512], phs[i, a][:])
                            oc = (a // 512) * 1024
                            nc.gpsimd.dma_start(out[:, oc:oc + 1024], ot[:])
            # banks not flushed early (EARLY_WB off, or no finalize step)
            for kf, a, b in regions:
                if (a, b) not in done:
                    ot = miscpool.tile([128, 2 * 512], f16,
                                       tag=f"otz_{a}", name=f"otz_{a}")
                    for i in range(2):
                        nc.vector.tensor_copy(
                            ot[:, i * 512:(i + 1) * 512], phs[i, a][:])
                    oc = (a // 512) * 1024
                    nc.gpsimd.dma_start(out[:, oc:oc + 1024], ot[:])
    return nc


def _prepare(msg, index, t, dim_size, W, b):
    """Host-side marshalling. Returns (in_maps, node_ids, schedule key)."""
    E, D = msg.shape
    counts = np.bincount(index, minlength=dim_size)
    order = np.lexsort((t, index))            # stable: primary index, secondary t
    msg_sorted = msg[order]                   # [E, D] grouped by node, t-ascending
    seg_starts = np.zeros(dim_size, np.int64)
    seg_starts[1:] = np.cumsum(counts)[:-1]

    nodesort = np.argsort(-counts, kind="stable")
    nz = nodesort[counts[nodesort] > 0]
    per_core = -(-len(nz) // N_CORES)
    assert per_core <= SLOTS, f"too many nodes per core: {per_core}"

    node_ids = np.full((N_CORES, SLOTS), -1, np.int64)
    for c in range(N_CORES):
        ids = nz[c::N_CORES]
        node_ids[c, :len(ids)] = ids
    cc = np.where(node_ids >= 0, counts[np.maximum(node_ids, 0)], 0)  # [8, SLOTS]

    Lmax = int(cc.max())
    n_k = tuple(int((cc > k).sum(axis=1).max()) for k in range(Lmax))

    # device handles steps k < K0; the tail (k >= K0, a small % of messages)
    # is folded into the per-node bias term on the host.  K0 stops where
    # fewer than TAIL_MIN slots are still live (weight DMA no longer pays
    # for itself); the SBUF guard keeps the no-reuse footprint bounded.
    K0 = Lmax
    while K0 > 1 and n_k[K0 - 1] < TAIL_MIN:
        K0 -= 1
    while K0 > 1 and (K0 * 512 + 2 * sum(n_k[:K0]) + 128 + 2 * SLOTS) > 85000:
        K0 -= 1
    Cdev = int(sum(n_k[:K0]))

    # column -> position in msg_sorted (or -1 = zero pad), k-major layout
    rowidx = np.full((N_CORES, Cdev), -1, np.int64)
    off = 0
    for k in range(K0):
        nk = n_k[k]
        nid = node_ids[:, :nk]
        ck = cc[:, :nk]
        active = k < ck
        pos = seg_starts[np.maximum(nid, 0)] + ck - 1 - k
        rowidx[:, off:off + nk] = np.where(active, pos, -1)
        off += nk

    # weights: powers of W in fp64, stored transposed (lhsT chunks).
    # wfull per k: cols [0,256) = (W^{k+1}).T rows 0:128 (j=0 chunk),
    #              cols [256,512) = rows 128:256 (j=1 chunk).
    Wd = W.astype(np.float64)
    bd = b.astype(np.float64)
    wfull = np.empty((128, K0 * 512), np.float16)
    s_table = np.zeros((Lmax + 1, D), np.float64)   # s_p = S_p b
    Wpows = []                                      # W^{k+1} (fp64), k = 0..Lmax-1
    P = Wd.copy()
    for k in range(Lmax):
        if k < K0:
            WT = P.T.astype(np.float16)             # (W^{k+1}).T
            wfull[:, k * 512:k * 512 + 256] = WT[:128, :]
            wfull[:, k * 512 + 256:(k + 1) * 512] = WT[128:, :]
        Wpows.append(P)
        s_table[k + 1] = Wd @ s_table[k] + bd
        P = P @ Wd

    # per-(core, slot) bias term: S_L b plus host-folded tail contributions
    bterm = s_table[cc]                              # [8, SLOTS, 256] fp64
    for k in range(K0, Lmax):
        nk = n_k[k]
        act = k < cc[:, :nk]                         # [8, nk]
        cs, ss = np.nonzero(act)
        pos = seg_starts[node_ids[cs, ss]] + cc[cs, ss] - 1 - k
        Y = msg_sorted[pos].astype(np.float64) @ Wpows[k].T
        bterm[cs, ss] += Y
    bterm16 = bterm.astype(np.float16)

    ident = np.zeros((128, 128), np.float16)
    np.fill_diagonal(ident, 1.0)

    banks = [0] + ([512] if n_k[0] > 512 else [])
    FB1 = 128 + 1024
    FB = FB1 * len(banks)
    Q = FB + K0 * 512 + 2 * Cdev
    in_maps = []
    for c in range(N_CORES):
        ri = rowidx[c]
        Mg = msg_sorted[np.maximum(ri, 0)].astype(np.float16)
        Mg[ri < 0] = 0.0                             # [Cdev, 256]
        hi = Mg[:, :128].T                           # [128, Cdev]
        lo = Mg[:, 128:].T
        mwb = np.empty((128, Q), np.float16)
        for bi, s in enumerate(banks):
            o = bi * FB1
            mwb[:, o:o + 128] = ident
            mwb[:, o + 128:o + 640] = bterm16[c, s:s + 512, :128].T
            mwb[:, o + 640:o + FB1] = bterm16[c, s:s + 512, 128:].T
        off = 0
        q = FB
        for k in range(K0):
            nk = n_k[k]
            mwb[:, q:q + 512] = wfull[:, k * 512:(k + 1) * 512]
            mwb[:, q + 512:q + 512 + nk] = hi[:, off:off + nk]
            mwb[:, q + 512 + nk:q + 512 + 2 * nk] = lo[:, off:off + nk]
            off += nk
            q += 512 + 2 * nk
        in_maps.append({"mw": mwb})
    return in_maps, node_ids, (K0, n_k[:K0], Cdev)


def _run(inputs: dict, trace: bool = False, **run_kwargs):
    msg = np.ascontiguousarray(np.asarray(inputs["msg"], dtype=np.float32))
    index = np.asarray(inputs["index"]).astype(np.int64)
    t = np.asarray(inputs["t"], dtype=np.float32)
    W = np.asarray(inputs["W"], dtype=np.float32)
    b = np.asarray(inputs["b"], dtype=np.float32)
    dim_size = int(inputs["dim_size"])

    in_maps, node_ids, key = _prepare(msg, index, t, dim_size, W, b)
    K0, n_k, Cdev = key
    if key not in _NC_CACHE:
        _NC_CACHE[key] = _build_nc(K0, n_k, Cdev)
    nc = _NC_CACHE[key]

    res = run_bass_kernel_spmd(nc, in_maps, list(range(N_CORES)),
                               trace=trace, **run_kwargs)

    hidden = np.zeros((dim_size, DIM), np.float32)
    for c in range(N_CORES):
        o = res.results[c]["out"].astype(np.float32)  # [128, 2*SLOTS]
        # out layout: [bank*1024 + i*512 + (slot - bank)] for bank in {0,512}
        hc = np.empty((SLOTS, DIM), np.float32)
        for bank in range(2):
            for i in range(2):
                blk = o[:, bank * 1024 + i * 512: bank * 1024 + (i + 1) * 512]
                hc[bank * 512:(bank + 1) * 512, i * 128:(i + 1) * 128] = blk.T
        valid = node_ids[c] >= 0
        hidden[node_ids[c][valid]] = hc[valid]
    return hidden, res


def kernel(**inputs) -> np.ndarray:
    hidden, _ = _run(inputs, trace=False)
    return hidden
